# revision 30
# baseline (speedup 1.0000x reference)
"""Additive (Bahdanau) attention on 8 trn2 NeuronCores — flipped sine-expansion.

Math per batch element b (one core each):
  logits[q,k] = sum_a w_a * tanh(x_qa + y_ka),  x = query@Wq^T, y = key@Wk^T + bias
  attn = softmax_k(logits);  out = attn @ value

tanh(z) ~ alpha*z + sum_h c_h sin(w_h z) with frequency set
  WA*{1,2,4,8} u WB*{1,2,3,4}  (fit on [-8.6, 8.6]).
Each sine factors sin(w(x+y)) = sin(wx)cos(wy)+cos(wx)sin(wy), so logits are
16 rank-128 fp16 matmuls accumulated in PSUM.  Logits are computed
TRANSPOSED ([k, q] with k on PSUM partitions):
  - the linear-in-y term alpha*(w@Wk)@kT becomes a per-partition Exp bias
    (no rank-1 PSUM opens); the per-q linear term cancels in softmax
  - no PE transposes in the out = attn@value epilogue (exp tiles are
    directly the stationary operand)
  - attn is written transposed as fp16; host transposes it back

Harmonic ladder in fp16 on the DVE.  Sine parts are stored DOWN-SCALED,
sigma_m = s_m / k_m, so the double-angle step s2=2*s*c becomes a plain
tensor_tensor (sigma2 = sigma*c), which runs in 2x packed mode — the
scalar_tensor_tensor form only has 1x uops.  The k_m factors fold into the
per-harmonic product scale w_a*c_h*k_h applied to the y-side slices
(scaled copies split between ScalarE and DVE).  b3 uses one Chebyshev step
with multiplier [4c1|2c1].  Seeds are ACT Sin at half/quarter angle.
"""

import numpy as np

import concourse.bass as bass
import concourse.tile as tile
from concourse import bacc, mybir
from concourse.bass_utils import run_bass_kernel_spmd

F32 = mybir.dt.float32
FP16 = mybir.dt.float16
AF = mybir.ActivationFunctionType
ALU = mybir.AluOpType

B, TQ, TK, DQ, DK, DV, A = 8, 512, 512, 512, 512, 512, 128
N_CORES = 8

WA, WB = 0.35, 0.52
RFIT = 8.6
HKEYS = ["a1", "a2", "a4", "a8", "b1", "b2", "b4"]
FREQS = [WA, 2 * WA, 4 * WA, 8 * WA, WB, 2 * WB, 4 * WB]
# sigma_m = s_m / KS[m]; c parts are stored true
KS = [1, 2, 4, 8, 2, 4, 8]
HALF_PI = float(np.pi / 2)

# quadrant slices of a level tile [sig_x | sig_y | c_x | c_y]
SX, SY, CX, CY = (slice(0, 512), slice(512, 1024),
                  slice(1024, 1536), slice(1536, 2048))
SH, CH = slice(0, 1024), slice(1024, 2048)  # sigma-half, c-half

_CACHE = {}


def _fit_params():
    if "fit" in _CACHE:
        return _CACHE["fit"]
    zg = np.linspace(-RFIT, RFIT, 6001)
    t = np.tanh(zg)
    Amat = np.stack([np.sin(w * zg) for w in FREQS] + [zg], 1)
    coef, *_ = np.linalg.lstsq(Amat, t, rcond=None)
    _CACHE["fit"] = (coef[:-1].astype(np.float64), float(coef[-1]))
    return _CACHE["fit"]


def build_nc():
    nc = bacc.Bacc(None, target_bir_lowering=False, debug=False)

    qT = nc.declare_dram_parameter("qT", [DQ, TQ], FP16, isOutput=False)
    kT = nc.declare_dram_parameter("kT", [DK, TK], FP16, isOutput=False)
    val = nc.declare_dram_parameter("value", [TK, DV], FP16, isOutput=False)
    WqT = nc.declare_dram_parameter("WqT", [DQ, A], FP16, isOutput=False)
    WkT = nc.declare_dram_parameter("WkT", [DK, A], FP16, isOutput=False)
    # f32 consts: c0=WA/2*b, c1=WA/2*b+pi/2, c2=WB/4*b, c3=WB/2*b, c4=pi/2
    cst_d = nc.declare_dram_parameter("cst", [A, 8], F32, isOutput=False)
    # fp16 consts: c0=1.0, c1=alpha*w_a
    cst16_d = nc.declare_dram_parameter("cst16", [A, 2], FP16, isOutput=False)
    wc_d = nc.declare_dram_parameter("wc", [A, 8], F32, isOutput=False)
    onesR_d = nc.declare_dram_parameter("onesR", [1, 128], FP16, isOutput=False)
    ident_d = nc.declare_dram_parameter("ident", [128, 128], FP16,
                                        isOutput=False)
    attnT_o = nc.declare_dram_parameter("attnT", [TK, TQ], FP16, isOutput=True)
    outN_o = nc.declare_dram_parameter("outN", [TQ, DV], FP16, isOutput=True)

    with tile.TileContext(nc) as tc:
        with (
            tc.tile_pool(name="pers", bufs=1) as pers,
            tc.tile_pool(name="tmp", bufs=3) as tmp_pool,
            tc.tile_pool(name="t2k", bufs=1) as t2k_pool,
            tc.tile_pool(name="scy", bufs=3) as scy_pool,
            tc.tile_pool(name="epi", bufs=4) as epi_pool,
            tc.tile_pool(name="psA", bufs=4, space="PSUM") as psA,
            tc.tile_pool(name="pout", bufs=4, space="PSUM") as pout_pool,
        ):
            # ---- persistent tiles ----
            cst = pers.tile([128, 8], F32, tag="cst")
            cst16 = pers.tile([128, 2], FP16, tag="cst16")
            wc = pers.tile([128, 8], F32, tag="wc")
            onesR = pers.tile([1, 128], FP16, tag="onesR")
            ident = pers.tile([128, 128], FP16, tag="ident")
            WkT_sb = pers.tile([128, DK // 128, A], FP16, tag="WkT_sb")
            WqT_sb = pers.tile([128, DQ // 128, A], FP16, tag="WqT_sb")
            kT_sb = pers.tile([128, DK // 128, TK], FP16, tag="kT_sb")
            qT_sb = pers.tile([128, DQ // 128, TQ], FP16, tag="qT_sb")
            value_sb = pers.tile([128, TK // 128, DV], FP16, tag="value_sb")
            y16 = pers.tile([128, TK], FP16, tag="y16")
            rk_sb = pers.tile([128, 4], F32, tag="rk_sb")
            # seeds [x | y]
            shA = pers.tile([128, 1024], FP16, tag="shA")
            qhB = pers.tile([128, 1024], FP16, tag="qhB")
            shB = pers.tile([128, 1024], FP16, tag="shB")
            chB = pers.tile([128, 1024], FP16, tag="chB")
            lev = {h: pers.tile([128, 2048], FP16, name=f"lev_{h}",
                                tag=f"lev_{h}")
                   for h in HKEYS}
            exp_t = [pers.tile([128, TQ], FP16, name=f"exp{g}", tag=f"exp{g}")
                     for g in range(4)]
            rec_row = pers.tile([1, TQ], FP16, tag="rec_row")
            tblw = pers.tile([128, 2], FP16, tag="tblw")

            # ---- input DMA (small chunks spread over queues/sequencers) ----
            kT_re = kT.rearrange("(c p) t -> p c t", p=128)
            qT_re = qT.rearrange("(c p) t -> p c t", p=128)
            val_re = val.rearrange("(c p) d -> p c d", p=128)
            WkT_re = WkT.rearrange("(c p) a -> p c a", p=128)
            WqT_re = WqT.rearrange("(c p) a -> p c a", p=128)
            # sync: consts, WkT+kT-left, then qT c2/c3 left, value
            nc.sync.dma_start(out=cst[:], in_=cst_d[:, :])
            for c in range(4):
                nc.sync.dma_start(out=kT_sb[:, c:c + 1, 0:256],
                                  in_=kT_re[:, c:c + 1, 0:256])
                nc.sync.dma_start(out=WkT_sb[:, c:c + 1, :],
                                  in_=WkT_re[:, c:c + 1, :])
            nc.sync.dma_start(out=qT_sb[:, 2:3, 0:256],
                              in_=qT_re[:, 2:3, 0:256])
            nc.sync.dma_start(out=qT_sb[:, 3:4, 0:256],
                              in_=qT_re[:, 3:4, 0:256])
            nc.sync.dma_start(out=wc[:], in_=wc_d[:, :])
            nc.sync.dma_start(out=value_sb[:, 0:2, :], in_=val_re[:, 0:2, :])
            # scalar: fp16 consts, kT right halves, qT c2/c3 right
            nc.scalar.dma_start(out=cst16[:], in_=cst16_d[:, :])
            for c in range(4):
                nc.scalar.dma_start(out=kT_sb[:, c:c + 1, 256:512],
                                    in_=kT_re[:, c:c + 1, 256:512])
            nc.scalar.dma_start(out=qT_sb[:, 2:3, 256:512],
                                in_=qT_re[:, 2:3, 256:512])
            nc.scalar.dma_start(out=qT_sb[:, 3:4, 256:512],
                                in_=qT_re[:, 3:4, 256:512])
            # gpsimd: WqT pairs, qT c0/c1 halves
            nc.gpsimd.dma_start(out=WqT_sb[:, 0:2, :], in_=WqT_re[:, 0:2, :])
            for c in range(2):
                nc.gpsimd.dma_start(out=qT_sb[:, c:c + 1, :],
                                    in_=qT_re[:, c:c + 1, :])
            nc.gpsimd.dma_start(out=WqT_sb[:, 2:4, :], in_=WqT_re[:, 2:4, :])
            nc.gpsimd.dma_start(out=onesR[:], in_=onesR_d[:, :])

            # ---- k projection + y seeds ----
            k_ps = psA.tile([128, TK], F32, tag="psA")
            for c in range(DK // 128):
                nc.tensor.matmul(k_ps[:], WkT_sb[:, c, :], kT_sb[:, c, :],
                                 start=(c == 0), stop=(c == DK // 128 - 1))
            nc.vector.tensor_copy(y16[:], k_ps[:])
            # tiny op to pull the Sin table load off the critical path
            nc.scalar.activation(tblw[:, 0:1], cst[:, 4:5], AF.Sin,
                                 bias=0.0, scale=1.0)
            nc.scalar.activation(shA[:, 512:1024], k_ps[:], AF.Sin,
                                 bias=cst[:, 0:1], scale=WA / 2)
            nc.scalar.activation(lev["a1"][:, SY], k_ps[:], AF.Sin,
                                 bias=cst[:, 5:6], scale=WA)
            nc.scalar.activation(qhB[:, 512:1024], k_ps[:], AF.Sin,
                                 bias=cst[:, 2:3], scale=WB / 4)
            nc.scalar.activation(shB[:, 512:1024], k_ps[:], AF.Sin,
                                 bias=cst[:, 3:4], scale=WB / 2)
            # low-priority input DMA behind the seeds on the scalar queue
            nc.scalar.dma_start(out=ident[:], in_=ident_d[:, :])
            nc.scalar.dma_start(out=value_sb[:, 2:4, :], in_=val_re[:, 2:4, :])

            # rk[k] = alpha * (w @ y_proj) as [128k, 1] per k-group
            rk_ps = psA.tile([128, 4], F32, tag="psA")
            for g in range(4):
                nc.tensor.matmul(rk_ps[:, g:g + 1],
                                 y16[:, g * 128:(g + 1) * 128],
                                 cst16[:, 1:2], start=True, stop=True,
                                 skip_group_check=True)
            nc.vector.tensor_copy(rk_sb[:], rk_ps[:])

            # ---- q projection + x seeds (A-family seeds first) ----
            q_ps = psA.tile([128, TQ], F32, tag="psA")
            for c in range(DQ // 128):
                nc.tensor.matmul(q_ps[:], WqT_sb[:, c, :], qT_sb[:, c, :],
                                 start=(c == 0), stop=(c == DQ // 128 - 1))
            nc.scalar.activation(shA[:, 0:512], q_ps[:], AF.Sin,
                                 bias=0.0, scale=WA / 2)
            nc.scalar.activation(lev["a1"][:, SX], q_ps[:], AF.Sin,
                                 bias=0.0, scale=WA)
            nc.scalar.activation(qhB[:, 0:512], q_ps[:], AF.Sin,
                                 bias=0.0, scale=WB / 4)
            nc.scalar.activation(shB[:, 0:512], q_ps[:], AF.Sin,
                                 bias=0.0, scale=WB / 2)

            # dense dummy matmuls to keep the PE busy (HAM warm) while
            # seeds/levels are generated
            dum_ps = psA.tile([128, 64], F32, tag="psA")
            for i in range(12):
                nc.tensor.matmul(dum_ps[:], WkT_sb[:, i % 4, :],
                                 WkT_sb[:, (i + 1) % 4, 0:64],
                                 start=True, stop=True, skip_group_check=True)

            plogs = [psA.tile([128, TQ], F32, name="plog", tag="psA")
                     for _ in range(4)]

            SXH, SYH = slice(0, 512), slice(512, 1024)

            def build_lev1(sl, ssl, csl):
                # family A: sigma1 = sin(wa t) direct (k=1); c1 = 1 - 2 shA^2
                uA = tmp_pool.tile([128, 512], FP16, tag="tmp")
                nc.vector.tensor_tensor(uA[:], shA[:, sl], shA[:, sl], ALU.mult)
                nc.vector.tensor_scalar(
                    lev["a1"][:, csl], uA[:], -2.0, 1.0, ALU.mult, ALU.add)

            def build_lev1B(sl, ssl, csl):
                uB = tmp_pool.tile([128, 512], FP16, tag="tmp")
                nc.vector.tensor_tensor(uB[:], qhB[:, sl], qhB[:, sl], ALU.mult)
                nc.vector.tensor_scalar(
                    chB[:, sl], uB[:], -2.0, 1.0, ALU.mult, ALU.add)
                nc.vector.tensor_tensor(lev["b1"][:, ssl], shB[:, sl],
                                        chB[:, sl], ALU.mult)
                uB2 = tmp_pool.tile([128, 512], FP16, tag="tmp")
                nc.vector.tensor_tensor(uB2[:], shB[:, sl], shB[:, sl],
                                        ALU.mult)
                nc.vector.tensor_scalar(
                    lev["b1"][:, csl], uB2[:], -2.0, 1.0, ALU.mult, ALU.add)

            def dbl(dst, src, ksrc):
                # sigma_2m = sigma_m * c_m ; c_2m = 1 - 2 k^2 sigma_m^2
                nc.vector.tensor_tensor(dst[:, SH], src[:, SH], src[:, CH],
                                        ALU.mult)
                u = tmp_pool.tile([128, 1024], FP16, tag="tmp")
                nc.vector.tensor_tensor(u[:], src[:, SH], src[:, SH], ALU.mult)
                nc.vector.tensor_scalar(
                    dst[:, CH], u[:], -2.0 * ksrc * ksrc, 1.0,
                    ALU.mult, ALU.add)

            def scy_make(h, s_eng, c_eng):
                s = scy_pool.tile([128, 1024], FP16, name=f"scy_{h}", tag="scy")
                hi = HKEYS.index(h)
                for half, src_sl, eng in ((slice(0, 512), SY, s_eng),
                                          (slice(512, 1024), CY, c_eng)):
                    if eng == "S":
                        nc.scalar.activation(s[:, half], lev[h][:, src_sl],
                                             AF.Copy, bias=0.0,
                                             scale=wc[:, hi:hi + 1])
                    elif eng == "G":
                        nc.gpsimd.tensor_scalar_mul(s[:, half],
                                                    lev[h][:, src_sl],
                                                    wc[:, hi:hi + 1])
                    else:
                        nc.vector.tensor_scalar_mul(s[:, half],
                                                    lev[h][:, src_sl],
                                                    wc[:, hi:hi + 1])
                return s

            def products(h, scy_t, first=False, last=False):
                for g in range(4):
                    nc.tensor.matmul(plogs[g][:],
                                     scy_t[:, g * 128:(g + 1) * 128],
                                     lev[h][:, CX], start=first, stop=False)
                    nc.tensor.matmul(plogs[g][:],
                                     scy_t[:, 512 + g * 128:512 + (g + 1) * 128],
                                     lev[h][:, SX], start=False, stop=last)

            # ---- ladder + products, interleaved ----
            build_lev1(SYH, SY, CY)
            for i in range(4):
                nc.tensor.matmul(dum_ps[:], shA[:, 512 + i * 128:640 + i * 128],
                                 shA[:, 512:576], start=True, stop=True,
                                 skip_group_check=True)
            build_lev1(SXH, SX, CX)
            s_a1 = scy_make("a1", "S", "V")
            products("a1", s_a1, first=True)

            dbl(lev["a2"], lev["a1"], 1)
            s_a2 = scy_make("a2", "S", "S")
            products("a2", s_a2)

            build_lev1B(SYH, SY, CY)
            build_lev1B(SXH, SX, CX)
            s_b1 = scy_make("b1", "S", "V")
            products("b1", s_b1)

            dbl(lev["b2"], lev["b1"], 2)
            s_b2 = scy_make("b2", "S", "S")
            products("b2", s_b2)

            dbl(lev["a4"], lev["a2"], 2)
            s_a4 = scy_make("a4", "V", "V")
            products("a4", s_a4)

            dbl(lev["a8"], lev["a4"], 4)
            s_a8 = scy_make("a8", "V", "V")
            products("a8", s_a8)

            dbl(lev["b4"], lev["b2"], 4)
            s_b4 = scy_make("b4", "V", "V")
            # pull the Exp table load off the critical path; reading s_b4
            # (the last scy) pins this late so the Copy table isn't evicted
            # while ScalarE scys still need it
            nc.scalar.activation(tblw[:, 1:2], s_b2[:, 0:1], AF.Exp,
                                 bias=0.0, scale=1.0)
            products("b4", s_b4, last=True)

            # ---- epilogue ----
            for g in range(4):
                nc.scalar.activation(exp_t[g][:], plogs[g][:], AF.Exp,
                                     bias=rk_sb[:, g:g + 1], scale=1.0)

            # out[q, d] = sum_k exp[k, q] value[k, d]; rowsums via ones column
            out_ps = [pout_pool.tile([128, DV], F32, name="out_ps", tag="out")
                      for _ in range(4)]
            rsq_ps = psA.tile([128, 4], F32, tag="psA")
            for gk in range(4):
                for gq in range(4):
                    nc.tensor.matmul(rsq_ps[:, gq:gq + 1],
                                     exp_t[gk][:, gq * 128:(gq + 1) * 128],
                                     cst16[:, 0:1],
                                     start=(gk == 0 and gq == 0),
                                     stop=(gk == 3 and gq == 3),
                                     skip_group_check=True)
                for gq in range(4):
                    nc.tensor.matmul(out_ps[gq][:],
                                     exp_t[gk][:, gq * 128:(gq + 1) * 128],
                                     value_sb[:, gk, :],
                                     start=(gk == 0), stop=(gk == 3))

            # one small reciprocal; broadcast it to a [128, TQ] tile
            recq = epi_pool.tile([128, 4], F32, tag="rec4")
            nc.vector.reciprocal(recq[:, :], rsq_ps[:, :])
            recq16 = epi_pool.tile([128, 4], FP16, tag="rec4")
            nc.vector.tensor_copy(recq16[:], recq[:])
            for g in range(4):
                rT = psA.tile([1, 128], FP16, name=f"recT{g}", tag="psA")
                nc.tensor.transpose(rT[:], recq16[:, g:g + 1], ident[:])
                nc.vector.tensor_copy(rec_row[0:1, g * 128:(g + 1) * 128],
                                      rT[:])
            recB_ps = psA.tile([128, TQ], F32, tag="psA")
            nc.tensor.matmul(recB_ps[:], onesR[:, :], rec_row[0:1, :],
                             start=True, stop=True)
            recB = epi_pool.tile([128, TQ], FP16, tag="recB")
            nc.vector.tensor_copy(recB[:], recB_ps[:])

            # attn tiles (normalized, fp16) + DMA out in halves
            for g in range(4):
                at = epi_pool.tile([128, TQ], FP16, name=f"at{g}", tag="at")
                nc.vector.tensor_tensor(at[:], exp_t[g][:], recB[:], ALU.mult)
                eng = (nc.sync, nc.gpsimd, nc.sync, nc.scalar)[g]
                eng.dma_start(out=attnT_o[g * 128:(g + 1) * 128, 0:256],
                              in_=at[:, 0:256])
                eng2 = (nc.gpsimd, nc.scalar, nc.scalar, nc.sync)[g]
                eng2.dma_start(out=attnT_o[g * 128:(g + 1) * 128, 256:512],
                               in_=at[:, 256:512])

            for gq in range(4):
                ot = epi_pool.tile([128, DV], FP16, name=f"ot{gq}", tag="ot")
                if gq % 2 == 0:
                    nc.vector.tensor_scalar_mul(ot[:], out_ps[gq][:],
                                                recq[:, gq:gq + 1])
                else:
                    nc.scalar.activation(ot[:], out_ps[gq][:], AF.Copy,
                                         bias=0.0, scale=recq[:, gq:gq + 1])
                eng = (nc.gpsimd, nc.scalar, nc.sync, nc.gpsimd)[gq]
                eng.dma_start(out=outN_o[gq * 128:(gq + 1) * 128, 0:256],
                              in_=ot[:, 0:256])
                eng2b = (nc.sync, nc.gpsimd, nc.gpsimd, nc.sync)[gq]
                eng2b.dma_start(out=outN_o[gq * 128:(gq + 1) * 128, 256:512],
                               in_=ot[:, 256:512])



    nc.compile()
    return nc


def _get_nc():
    if "nc" not in _CACHE:
        _CACHE["nc"] = build_nc()
    return _CACHE["nc"]


def make_in_maps(query, key, value, Wq, Wk, bias, w_w, **_):
    coeffs, alpha = _fit_params()
    w = np.asarray(w_w, dtype=np.float64).reshape(A)
    b = np.asarray(bias, dtype=np.float64).reshape(A)
    WqT = np.ascontiguousarray(Wq.T).astype(np.float16)
    WkT = np.ascontiguousarray(Wk.T).astype(np.float16)
    cst = np.zeros((A, 8), dtype=np.float32)
    cst[:, 0] = WA / 2 * b
    cst[:, 1] = WA / 2 * b + HALF_PI
    cst[:, 2] = WB / 4 * b
    cst[:, 3] = WB / 2 * b
    cst[:, 4] = HALF_PI
    cst[:, 5] = WA * b
    cst16 = np.zeros((A, 2), dtype=np.float16)
    cst16[:, 0] = 1.0
    cst16[:, 1] = (alpha * w).astype(np.float16)
    # per-harmonic product scale, with the sigma ladder k_h folded in
    wc = np.zeros((A, 8), dtype=np.float32)
    wc[:, :len(KS)] = (w[:, None]
                       * (coeffs * np.array(KS, np.float64))[None, :])
    onesR = np.ones((1, 128), dtype=np.float16)
    ident = np.eye(128, dtype=np.float16)
    in_maps = []
    for bb in range(B):
        in_maps.append({
            "qT": np.ascontiguousarray(query[bb].T).astype(np.float16),
            "kT": np.ascontiguousarray(key[bb].T).astype(np.float16),
            "value": np.ascontiguousarray(value[bb]).astype(np.float16),
            "WqT": WqT,
            "WkT": WkT,
            "cst": cst,
            "cst16": cst16,
            "wc": wc,
            "onesR": onesR,
            "ident": ident,
        })
    return in_maps


def run(inputs, trace=False, **kwargs):
    nc = _get_nc()
    in_maps = make_in_maps(**{k: np.asarray(v) for k, v in inputs.items()})
    res = run_bass_kernel_spmd(
        nc, in_maps, list(range(N_CORES)), trace=trace, **kwargs
    )
    output = np.stack([res.results[bb]["outN"].astype(np.float32)
                       for bb in range(B)])
    attn = np.stack([
        np.ascontiguousarray(res.results[bb]["attnT"].T).astype(np.float32)
        for bb in range(B)])
    return (output, attn), res


def kernel(**inputs):
    (output, attn), _ = run(inputs)
    return output, attn


# revision 31
# speedup vs baseline: 1.0186x; 1.0186x over previous
"""Additive (Bahdanau) attention on 8 trn2 NeuronCores — flipped sine-expansion.

Math per batch element b (one core each):
  logits[q,k] = sum_a w_a * tanh(x_qa + y_ka),  x = query@Wq^T, y = key@Wk^T + bias
  attn = softmax_k(logits);  out = attn @ value

tanh(z) ~ alpha*z + sum_h c_h sin(w_h z) with frequency set
  WA*{1,2,4,8} u WB*{1,2,3,4}  (fit on [-8.6, 8.6]).
Each sine factors sin(w(x+y)) = sin(wx)cos(wy)+cos(wx)sin(wy), so logits are
16 rank-128 fp16 matmuls accumulated in PSUM.  Logits are computed
TRANSPOSED ([k, q] with k on PSUM partitions):
  - the linear-in-y term alpha*(w@Wk)@kT becomes a per-partition Exp bias
    (no rank-1 PSUM opens); the per-q linear term cancels in softmax
  - no PE transposes in the out = attn@value epilogue (exp tiles are
    directly the stationary operand)
  - attn is written transposed as fp16; host transposes it back

Harmonic ladder in fp16 on the DVE.  Sine parts are stored DOWN-SCALED,
sigma_m = s_m / k_m, so the double-angle step s2=2*s*c becomes a plain
tensor_tensor (sigma2 = sigma*c), which runs in 2x packed mode — the
scalar_tensor_tensor form only has 1x uops.  The k_m factors fold into the
per-harmonic product scale w_a*c_h*k_h applied to the y-side slices
(scaled copies split between ScalarE and DVE).  b3 uses one Chebyshev step
with multiplier [4c1|2c1].  Seeds are ACT Sin at half/quarter angle.
"""

import numpy as np

import concourse.bass as bass
import concourse.tile as tile
from concourse import bacc, mybir
from concourse.bass_utils import run_bass_kernel_spmd

F32 = mybir.dt.float32
FP16 = mybir.dt.float16
AF = mybir.ActivationFunctionType
ALU = mybir.AluOpType

B, TQ, TK, DQ, DK, DV, A = 8, 512, 512, 512, 512, 512, 128
N_CORES = 8

WA, WB = 0.35, 0.52
RFIT = 8.6
HKEYS = ["a1", "a2", "a4", "a8", "b1", "b2", "b4"]
FREQS = [WA, 2 * WA, 4 * WA, 8 * WA, WB, 2 * WB, 4 * WB]
# sigma_m = s_m / KS[m]; c parts are stored true
KS = [1, 2, 4, 8, 2, 4, 8]
HALF_PI = float(np.pi / 2)

# quadrant slices of a level tile [sig_x | sig_y | c_x | c_y]
SX, SY, CX, CY = (slice(0, 512), slice(512, 1024),
                  slice(1024, 1536), slice(1536, 2048))
SH, CH = slice(0, 1024), slice(1024, 2048)  # sigma-half, c-half

_CACHE = {}


def _fit_params():
    if "fit" in _CACHE:
        return _CACHE["fit"]
    zg = np.linspace(-RFIT, RFIT, 6001)
    t = np.tanh(zg)
    Amat = np.stack([np.sin(w * zg) for w in FREQS] + [zg], 1)
    coef, *_ = np.linalg.lstsq(Amat, t, rcond=None)
    _CACHE["fit"] = (coef[:-1].astype(np.float64), float(coef[-1]))
    return _CACHE["fit"]


def build_nc():
    nc = bacc.Bacc(None, target_bir_lowering=False, debug=False)

    qT = nc.declare_dram_parameter("qT", [DQ, TQ], FP16, isOutput=False)
    kT = nc.declare_dram_parameter("kT", [DK, TK], FP16, isOutput=False)
    val = nc.declare_dram_parameter("value", [TK, DV], FP16, isOutput=False)
    WqT = nc.declare_dram_parameter("WqT", [DQ, A], FP16, isOutput=False)
    WkT = nc.declare_dram_parameter("WkT", [DK, A], FP16, isOutput=False)
    # f32 consts: c0=WA/2*b, c1=WA/2*b+pi/2, c2=WB/4*b, c3=WB/2*b, c4=pi/2
    cst_d = nc.declare_dram_parameter("cst", [A, 8], F32, isOutput=False)
    # fp16 consts: c0=1.0, c1=alpha*w_a
    cst16_d = nc.declare_dram_parameter("cst16", [A, 2], FP16, isOutput=False)
    wc_d = nc.declare_dram_parameter("wc", [A, 8], F32, isOutput=False)
    onesR_d = nc.declare_dram_parameter("onesR", [1, 128], FP16, isOutput=False)
    ident_d = nc.declare_dram_parameter("ident", [128, 128], FP16,
                                        isOutput=False)
    attnT_o = nc.declare_dram_parameter("attnT", [TK, TQ], FP16, isOutput=True)
    outN_o = nc.declare_dram_parameter("outN", [TQ, DV], FP16, isOutput=True)

    with tile.TileContext(nc) as tc:
        with (
            tc.tile_pool(name="pers", bufs=1) as pers,
            tc.tile_pool(name="tmp", bufs=3) as tmp_pool,
            tc.tile_pool(name="t2k", bufs=1) as t2k_pool,
            tc.tile_pool(name="scy", bufs=3) as scy_pool,
            tc.tile_pool(name="epi", bufs=4) as epi_pool,
            tc.tile_pool(name="psA", bufs=4, space="PSUM") as psA,
            tc.tile_pool(name="pout", bufs=4, space="PSUM") as pout_pool,
        ):
            # ---- persistent tiles ----
            cst = pers.tile([128, 8], F32, tag="cst")
            cst16 = pers.tile([128, 2], FP16, tag="cst16")
            wc = pers.tile([128, 8], F32, tag="wc")
            onesR = pers.tile([1, 128], FP16, tag="onesR")
            ident = pers.tile([128, 128], FP16, tag="ident")
            WkT_sb = pers.tile([128, DK // 128, A], FP16, tag="WkT_sb")
            WqT_sb = pers.tile([128, DQ // 128, A], FP16, tag="WqT_sb")
            kT_sb = pers.tile([128, DK // 128, TK], FP16, tag="kT_sb")
            qT_sb = pers.tile([128, DQ // 128, TQ], FP16, tag="qT_sb")
            value_sb = pers.tile([128, TK // 128, DV], FP16, tag="value_sb")
            y16 = pers.tile([128, TK], FP16, tag="y16")
            rk_sb = pers.tile([128, 4], F32, tag="rk_sb")
            # seeds [x | y]
            shA = pers.tile([128, 1024], FP16, tag="shA")
            qhB = pers.tile([128, 1024], FP16, tag="qhB")
            shB = pers.tile([128, 1024], FP16, tag="shB")
            chB = pers.tile([128, 1024], FP16, tag="chB")
            lev = {h: pers.tile([128, 2048], FP16, name=f"lev_{h}",
                                tag=f"lev_{h}")
                   for h in HKEYS}
            exp_t = [pers.tile([128, TQ], FP16, name=f"exp{g}", tag=f"exp{g}")
                     for g in range(4)]
            rec_row = pers.tile([1, TQ], FP16, tag="rec_row")
            tblw = pers.tile([128, 2], FP16, tag="tblw")

            # ---- input DMA (small chunks spread over queues/sequencers) ----
            kT_re = kT.rearrange("(c p) t -> p c t", p=128)
            qT_re = qT.rearrange("(c p) t -> p c t", p=128)
            val_re = val.rearrange("(c p) d -> p c d", p=128)
            WkT_re = WkT.rearrange("(c p) a -> p c a", p=128)
            WqT_re = WqT.rearrange("(c p) a -> p c a", p=128)
            # sync: consts, WkT+kT-left, then qT c2/c3 left, value
            nc.sync.dma_start(out=cst[:], in_=cst_d[:, :])
            for c in range(4):
                nc.sync.dma_start(out=kT_sb[:, c:c + 1, 0:256],
                                  in_=kT_re[:, c:c + 1, 0:256])
                nc.sync.dma_start(out=WkT_sb[:, c:c + 1, :],
                                  in_=WkT_re[:, c:c + 1, :])
            nc.sync.dma_start(out=qT_sb[:, 2:3, 0:256],
                              in_=qT_re[:, 2:3, 0:256])
            nc.sync.dma_start(out=qT_sb[:, 3:4, 0:256],
                              in_=qT_re[:, 3:4, 0:256])
            nc.sync.dma_start(out=wc[:], in_=wc_d[:, :])
            nc.sync.dma_start(out=value_sb[:, 0:2, :], in_=val_re[:, 0:2, :])
            # scalar: fp16 consts, kT right halves, qT c2/c3 right
            nc.scalar.dma_start(out=cst16[:], in_=cst16_d[:, :])
            for c in range(4):
                nc.scalar.dma_start(out=kT_sb[:, c:c + 1, 256:512],
                                    in_=kT_re[:, c:c + 1, 256:512])
            nc.scalar.dma_start(out=qT_sb[:, 2:3, 256:512],
                                in_=qT_re[:, 2:3, 256:512])
            nc.scalar.dma_start(out=qT_sb[:, 3:4, 256:512],
                                in_=qT_re[:, 3:4, 256:512])
            # gpsimd: WqT pairs, qT c0/c1 halves
            nc.gpsimd.dma_start(out=WqT_sb[:, 0:2, :], in_=WqT_re[:, 0:2, :])
            for c in range(2):
                nc.gpsimd.dma_start(out=qT_sb[:, c:c + 1, :],
                                    in_=qT_re[:, c:c + 1, :])
            nc.gpsimd.dma_start(out=WqT_sb[:, 2:4, :], in_=WqT_re[:, 2:4, :])
            nc.gpsimd.dma_start(out=onesR[:], in_=onesR_d[:, :])

            # ---- k projection + y seeds ----
            k_ps = psA.tile([128, TK], F32, tag="psA")
            for c in range(DK // 128):
                nc.tensor.matmul(k_ps[:], WkT_sb[:, c, :], kT_sb[:, c, :],
                                 start=(c == 0), stop=(c == DK // 128 - 1))
            nc.vector.tensor_copy(y16[:], k_ps[:])
            # tiny op to pull the Sin table load off the critical path
            nc.scalar.activation(tblw[:, 0:1], cst[:, 4:5], AF.Sin,
                                 bias=0.0, scale=1.0)
            nc.scalar.activation(shA[:, 512:1024], k_ps[:], AF.Sin,
                                 bias=cst[:, 0:1], scale=WA / 2)
            nc.scalar.activation(lev["a1"][:, SY], k_ps[:], AF.Sin,
                                 bias=cst[:, 5:6], scale=WA)
            nc.scalar.activation(qhB[:, 512:1024], k_ps[:], AF.Sin,
                                 bias=cst[:, 2:3], scale=WB / 4)
            nc.scalar.activation(shB[:, 512:1024], k_ps[:], AF.Sin,
                                 bias=cst[:, 3:4], scale=WB / 2)
            # low-priority input DMA behind the seeds on the scalar queue
            nc.scalar.dma_start(out=ident[:], in_=ident_d[:, :])
            nc.scalar.dma_start(out=value_sb[:, 2:4, :], in_=val_re[:, 2:4, :])

            # rk[k] = alpha * (w @ y_proj) as [128k, 1] per k-group
            rk_ps = psA.tile([128, 4], F32, tag="psA")
            for g in range(4):
                nc.tensor.matmul(rk_ps[:, g:g + 1],
                                 y16[:, g * 128:(g + 1) * 128],
                                 cst16[:, 1:2], start=True, stop=True,
                                 skip_group_check=True)
            nc.vector.tensor_copy(rk_sb[:], rk_ps[:])

            # ---- q projection + x seeds (A-family seeds first) ----
            q_ps = psA.tile([128, TQ], F32, tag="psA")
            for c in range(DQ // 128):
                nc.tensor.matmul(q_ps[:], WqT_sb[:, c, :], qT_sb[:, c, :],
                                 start=(c == 0), stop=(c == DQ // 128 - 1))
            nc.scalar.activation(shA[:, 0:512], q_ps[:], AF.Sin,
                                 bias=0.0, scale=WA / 2)
            nc.scalar.activation(lev["a1"][:, SX], q_ps[:], AF.Sin,
                                 bias=0.0, scale=WA)
            nc.scalar.activation(qhB[:, 0:512], q_ps[:], AF.Sin,
                                 bias=0.0, scale=WB / 4)
            nc.scalar.activation(shB[:, 0:512], q_ps[:], AF.Sin,
                                 bias=0.0, scale=WB / 2)

            # dense dummy matmuls to keep the PE busy (HAM warm) while
            # seeds/levels are generated
            dum_ps = psA.tile([128, 64], F32, tag="psA")
            for i in range(12):
                nc.tensor.matmul(dum_ps[:], WkT_sb[:, i % 4, :],
                                 WkT_sb[:, (i + 1) % 4, 0:64],
                                 start=True, stop=True, skip_group_check=True)

            plogs = [psA.tile([128, TQ], F32, name="plog", tag="psA")
                     for _ in range(4)]

            SXH, SYH = slice(0, 512), slice(512, 1024)

            def build_lev1(sl, ssl, csl):
                # family A: sigma1 = sin(wa t) direct (k=1); c1 = 1 - 2 shA^2
                uA = tmp_pool.tile([128, 512], FP16, tag="tmp")
                nc.vector.tensor_tensor(uA[:], shA[:, sl], shA[:, sl], ALU.mult)
                nc.vector.tensor_scalar(
                    lev["a1"][:, csl], uA[:], -2.0, 1.0, ALU.mult, ALU.add)

            def build_lev1B(sl, ssl, csl):
                uB = tmp_pool.tile([128, 512], FP16, tag="tmp")
                nc.vector.tensor_tensor(uB[:], qhB[:, sl], qhB[:, sl], ALU.mult)
                nc.vector.tensor_scalar(
                    chB[:, sl], uB[:], -2.0, 1.0, ALU.mult, ALU.add)
                nc.vector.tensor_tensor(lev["b1"][:, ssl], shB[:, sl],
                                        chB[:, sl], ALU.mult)
                uB2 = tmp_pool.tile([128, 512], FP16, tag="tmp")
                nc.vector.tensor_tensor(uB2[:], shB[:, sl], shB[:, sl],
                                        ALU.mult)
                nc.vector.tensor_scalar(
                    lev["b1"][:, csl], uB2[:], -2.0, 1.0, ALU.mult, ALU.add)

            def dbl(dst, src, ksrc):
                # sigma_2m = sigma_m * c_m ; c_2m = 1 - 2 k^2 sigma_m^2
                nc.vector.tensor_tensor(dst[:, SH], src[:, SH], src[:, CH],
                                        ALU.mult)
                u = tmp_pool.tile([128, 1024], FP16, tag="tmp")
                nc.vector.tensor_tensor(u[:], src[:, SH], src[:, SH], ALU.mult)
                nc.vector.tensor_scalar(
                    dst[:, CH], u[:], -2.0 * ksrc * ksrc, 1.0,
                    ALU.mult, ALU.add)

            def scy_make(h, s_eng, c_eng):
                s = scy_pool.tile([128, 1024], FP16, name=f"scy_{h}", tag="scy")
                hi = HKEYS.index(h)
                for half, src_sl, eng in ((slice(0, 512), SY, s_eng),
                                          (slice(512, 1024), CY, c_eng)):
                    if eng == "S":
                        nc.scalar.activation(s[:, half], lev[h][:, src_sl],
                                             AF.Copy, bias=0.0,
                                             scale=wc[:, hi:hi + 1])
                    elif eng == "G":
                        nc.gpsimd.tensor_scalar_mul(s[:, half],
                                                    lev[h][:, src_sl],
                                                    wc[:, hi:hi + 1])
                    else:
                        nc.vector.tensor_scalar_mul(s[:, half],
                                                    lev[h][:, src_sl],
                                                    wc[:, hi:hi + 1])
                return s

            def products(h, scy_t, first=False, last=False):
                for g in range(4):
                    nc.tensor.matmul(plogs[g][:],
                                     scy_t[:, g * 128:(g + 1) * 128],
                                     lev[h][:, CX], start=first, stop=False)
                    nc.tensor.matmul(plogs[g][:],
                                     scy_t[:, 512 + g * 128:512 + (g + 1) * 128],
                                     lev[h][:, SX], start=False, stop=last)

            # ---- ladder + products, interleaved ----
            build_lev1(SYH, SY, CY)
            for i in range(4):
                nc.tensor.matmul(dum_ps[:], shA[:, 512 + i * 128:640 + i * 128],
                                 shA[:, 512:576], start=True, stop=True,
                                 skip_group_check=True)
            build_lev1(SXH, SX, CX)
            s_a1 = scy_make("a1", "V", "V")
            products("a1", s_a1, first=True)

            dbl(lev["a2"], lev["a1"], 1)
            s_a2 = scy_make("a2", "S", "V")
            products("a2", s_a2)

            build_lev1B(SYH, SY, CY)
            build_lev1B(SXH, SX, CX)
            s_b1 = scy_make("b1", "S", "V")
            products("b1", s_b1)

            dbl(lev["b2"], lev["b1"], 2)
            s_b2 = scy_make("b2", "S", "S")
            products("b2", s_b2)

            dbl(lev["a4"], lev["a2"], 2)
            s_a4 = scy_make("a4", "V", "V")
            products("a4", s_a4)

            dbl(lev["a8"], lev["a4"], 4)
            s_a8 = scy_make("a8", "V", "V")
            products("a8", s_a8)

            dbl(lev["b4"], lev["b2"], 4)
            s_b4 = scy_make("b4", "V", "V")
            # pull the Exp table load off the critical path; reading s_b4
            # (the last scy) pins this late so the Copy table isn't evicted
            # while ScalarE scys still need it
            nc.scalar.activation(tblw[:, 1:2], s_b2[:, 0:1], AF.Exp,
                                 bias=0.0, scale=1.0)
            products("b4", s_b4, last=True)

            # ---- epilogue ----
            for g in range(4):
                nc.scalar.activation(exp_t[g][:], plogs[g][:], AF.Exp,
                                     bias=rk_sb[:, g:g + 1], scale=1.0)

            # out[q, d] = sum_k exp[k, q] value[k, d]; rowsums via ones column
            out_ps = [pout_pool.tile([128, DV], F32, name="out_ps", tag="out")
                      for _ in range(4)]
            rsq_ps = psA.tile([128, 4], F32, tag="psA")
            for gk in range(4):
                for gq in range(4):
                    nc.tensor.matmul(rsq_ps[:, gq:gq + 1],
                                     exp_t[gk][:, gq * 128:(gq + 1) * 128],
                                     cst16[:, 0:1],
                                     start=(gk == 0 and gq == 0),
                                     stop=(gk == 3 and gq == 3),
                                     skip_group_check=True)
                for gq in range(4):
                    nc.tensor.matmul(out_ps[gq][:],
                                     exp_t[gk][:, gq * 128:(gq + 1) * 128],
                                     value_sb[:, gk, :],
                                     start=(gk == 0), stop=(gk == 3))

            # one small reciprocal; broadcast it to a [128, TQ] tile
            recq = epi_pool.tile([128, 4], F32, tag="rec4")
            nc.vector.reciprocal(recq[:, :], rsq_ps[:, :])
            recq16 = epi_pool.tile([128, 4], FP16, tag="rec4")
            nc.vector.tensor_copy(recq16[:], recq[:])
            for g in range(4):
                rT = psA.tile([1, 128], FP16, name=f"recT{g}", tag="psA")
                nc.tensor.transpose(rT[:], recq16[:, g:g + 1], ident[:])
                nc.vector.tensor_copy(rec_row[0:1, g * 128:(g + 1) * 128],
                                      rT[:])
            recB_ps = psA.tile([128, TQ], F32, tag="psA")
            nc.tensor.matmul(recB_ps[:], onesR[:, :], rec_row[0:1, :],
                             start=True, stop=True)
            recB = epi_pool.tile([128, TQ], FP16, tag="recB")
            nc.vector.tensor_copy(recB[:], recB_ps[:])

            # attn tiles (normalized, fp16) + DMA out in halves
            for g in range(4):
                at = epi_pool.tile([128, TQ], FP16, name=f"at{g}", tag="at")
                nc.vector.tensor_tensor(at[:], exp_t[g][:], recB[:], ALU.mult)
                eng = (nc.sync, nc.gpsimd, nc.sync, nc.scalar)[g]
                eng.dma_start(out=attnT_o[g * 128:(g + 1) * 128, 0:256],
                              in_=at[:, 0:256])
                eng2 = (nc.gpsimd, nc.scalar, nc.scalar, nc.sync)[g]
                eng2.dma_start(out=attnT_o[g * 128:(g + 1) * 128, 256:512],
                               in_=at[:, 256:512])

            for gq in range(4):
                ot = epi_pool.tile([128, DV], FP16, name=f"ot{gq}", tag="ot")
                if gq % 2 == 0:
                    nc.vector.tensor_scalar_mul(ot[:], out_ps[gq][:],
                                                recq[:, gq:gq + 1])
                else:
                    nc.scalar.activation(ot[:], out_ps[gq][:], AF.Copy,
                                         bias=0.0, scale=recq[:, gq:gq + 1])
                eng = (nc.gpsimd, nc.scalar, nc.sync, nc.gpsimd)[gq]
                eng.dma_start(out=outN_o[gq * 128:(gq + 1) * 128, 0:256],
                              in_=ot[:, 0:256])
                eng2b = (nc.sync, nc.gpsimd, nc.gpsimd, nc.sync)[gq]
                eng2b.dma_start(out=outN_o[gq * 128:(gq + 1) * 128, 256:512],
                               in_=ot[:, 256:512])



    nc.compile()
    return nc


def _get_nc():
    if "nc" not in _CACHE:
        _CACHE["nc"] = build_nc()
    return _CACHE["nc"]


def make_in_maps(query, key, value, Wq, Wk, bias, w_w, **_):
    coeffs, alpha = _fit_params()
    w = np.asarray(w_w, dtype=np.float64).reshape(A)
    b = np.asarray(bias, dtype=np.float64).reshape(A)
    WqT = np.ascontiguousarray(Wq.T).astype(np.float16)
    WkT = np.ascontiguousarray(Wk.T).astype(np.float16)
    cst = np.zeros((A, 8), dtype=np.float32)
    cst[:, 0] = WA / 2 * b
    cst[:, 1] = WA / 2 * b + HALF_PI
    cst[:, 2] = WB / 4 * b
    cst[:, 3] = WB / 2 * b
    cst[:, 4] = HALF_PI
    cst[:, 5] = WA * b
    cst16 = np.zeros((A, 2), dtype=np.float16)
    cst16[:, 0] = 1.0
    cst16[:, 1] = (alpha * w).astype(np.float16)
    # per-harmonic product scale, with the sigma ladder k_h folded in
    wc = np.zeros((A, 8), dtype=np.float32)
    wc[:, :len(KS)] = (w[:, None]
                       * (coeffs * np.array(KS, np.float64))[None, :])
    onesR = np.ones((1, 128), dtype=np.float16)
    ident = np.eye(128, dtype=np.float16)
    in_maps = []
    for bb in range(B):
        in_maps.append({
            "qT": np.ascontiguousarray(query[bb].T).astype(np.float16),
            "kT": np.ascontiguousarray(key[bb].T).astype(np.float16),
            "value": np.ascontiguousarray(value[bb]).astype(np.float16),
            "WqT": WqT,
            "WkT": WkT,
            "cst": cst,
            "cst16": cst16,
            "wc": wc,
            "onesR": onesR,
            "ident": ident,
        })
    return in_maps


def run(inputs, trace=False, **kwargs):
    nc = _get_nc()
    in_maps = make_in_maps(**{k: np.asarray(v) for k, v in inputs.items()})
    res = run_bass_kernel_spmd(
        nc, in_maps, list(range(N_CORES)), trace=trace, **kwargs
    )
    output = np.stack([res.results[bb]["outN"].astype(np.float32)
                       for bb in range(B)])
    attn = np.stack([
        np.ascontiguousarray(res.results[bb]["attnT"].T).astype(np.float32)
        for bb in range(B)])
    return (output, attn), res


def kernel(**inputs):
    (output, attn), _ = run(inputs)
    return output, attn


# revision 32
# speedup vs baseline: 1.0544x; 1.0351x over previous
"""Additive (Bahdanau) attention on 8 trn2 NeuronCores — flipped sine-expansion.

Math per batch element b (one core each):
  logits[q,k] = sum_a w_a * tanh(x_qa + y_ka),  x = query@Wq^T, y = key@Wk^T + bias
  attn = softmax_k(logits);  out = attn @ value

tanh(z) ~ alpha*z + sum_h c_h sin(w_h z) with frequency set
  WA*{1,2,4,8} u WB*{1,2,3,4}  (fit on [-8.6, 8.6]).
Each sine factors sin(w(x+y)) = sin(wx)cos(wy)+cos(wx)sin(wy), so logits are
16 rank-128 fp16 matmuls accumulated in PSUM.  Logits are computed
TRANSPOSED ([k, q] with k on PSUM partitions):
  - the linear-in-y term alpha*(w@Wk)@kT becomes a per-partition Exp bias
    (no rank-1 PSUM opens); the per-q linear term cancels in softmax
  - no PE transposes in the out = attn@value epilogue (exp tiles are
    directly the stationary operand)
  - attn is written transposed as fp16; host transposes it back

Harmonic ladder in fp16 on the DVE.  Sine parts are stored DOWN-SCALED,
sigma_m = s_m / k_m, so the double-angle step s2=2*s*c becomes a plain
tensor_tensor (sigma2 = sigma*c), which runs in 2x packed mode — the
scalar_tensor_tensor form only has 1x uops.  The k_m factors fold into the
per-harmonic product scale w_a*c_h*k_h applied to the y-side slices
(scaled copies split between ScalarE and DVE).  b3 uses one Chebyshev step
with multiplier [4c1|2c1].  Seeds are ACT Sin at half/quarter angle.
"""

import numpy as np

import concourse.bass as bass
import concourse.tile as tile
from concourse import bacc, mybir
from concourse.bass_utils import run_bass_kernel_spmd

F32 = mybir.dt.float32
FP16 = mybir.dt.float16
AF = mybir.ActivationFunctionType
ALU = mybir.AluOpType

B, TQ, TK, DQ, DK, DV, A = 8, 512, 512, 512, 512, 512, 128
N_CORES = 8

WA, WB = 0.35, 0.52
RFIT = 8.6
HKEYS = ["a1", "a2", "a4", "a8", "b1", "b2", "b4"]
FREQS = [WA, 2 * WA, 4 * WA, 8 * WA, WB, 2 * WB, 4 * WB]
# sigma_m = s_m / KS[m]; c parts are stored true
KS = [1, 2, 4, 8, 2, 4, 8]
HALF_PI = float(np.pi / 2)

# quadrant slices of a level tile [sig_x | sig_y | c_x | c_y]
SX, SY, CX, CY = (slice(0, 512), slice(512, 1024),
                  slice(1024, 1536), slice(1536, 2048))
SH, CH = slice(0, 1024), slice(1024, 2048)  # sigma-half, c-half

_CACHE = {}


def _fit_params():
    if "fit" in _CACHE:
        return _CACHE["fit"]
    zg = np.linspace(-RFIT, RFIT, 6001)
    t = np.tanh(zg)
    Amat = np.stack([np.sin(w * zg) for w in FREQS] + [zg], 1)
    coef, *_ = np.linalg.lstsq(Amat, t, rcond=None)
    _CACHE["fit"] = (coef[:-1].astype(np.float64), float(coef[-1]))
    return _CACHE["fit"]


def build_nc():
    nc = bacc.Bacc(None, target_bir_lowering=False, debug=False)

    qT = nc.declare_dram_parameter("qT", [DQ, TQ], FP16, isOutput=False)
    kT = nc.declare_dram_parameter("kT", [DK, TK], FP16, isOutput=False)
    val = nc.declare_dram_parameter("value", [TK, DV], FP16, isOutput=False)
    WqT = nc.declare_dram_parameter("WqT", [DQ, A], FP16, isOutput=False)
    WkT = nc.declare_dram_parameter("WkT", [DK, A], FP16, isOutput=False)
    # f32 consts: c0=WA/2*b, c1=WA/2*b+pi/2, c2=WB/4*b, c3=WB/2*b, c4=pi/2
    cst_d = nc.declare_dram_parameter("cst", [A, 8], F32, isOutput=False)
    # fp16 consts: c0=1.0, c1=alpha*w_a
    cst16_d = nc.declare_dram_parameter("cst16", [A, 2], FP16, isOutput=False)
    wc_d = nc.declare_dram_parameter("wc", [A, 8], F32, isOutput=False)
    onesR_d = nc.declare_dram_parameter("onesR", [1, 128], FP16, isOutput=False)
    ident_d = nc.declare_dram_parameter("ident", [128, 128], FP16,
                                        isOutput=False)
    attnT_o = nc.declare_dram_parameter("attnT", [TK, TQ], FP16, isOutput=True)
    outN_o = nc.declare_dram_parameter("outN", [TQ, DV], FP16, isOutput=True)

    with tile.TileContext(nc) as tc:
        with (
            tc.tile_pool(name="pers", bufs=1) as pers,
            tc.tile_pool(name="tmp", bufs=3) as tmp_pool,
            tc.tile_pool(name="t2k", bufs=1) as t2k_pool,
            tc.tile_pool(name="scy", bufs=3) as scy_pool,
            tc.tile_pool(name="epi", bufs=4) as epi_pool,
            tc.tile_pool(name="psA", bufs=4, space="PSUM") as psA,
            tc.tile_pool(name="pout", bufs=4, space="PSUM") as pout_pool,
        ):
            # ---- persistent tiles ----
            cst = pers.tile([128, 8], F32, tag="cst")
            cst16 = pers.tile([128, 2], FP16, tag="cst16")
            wc = pers.tile([128, 8], F32, tag="wc")
            onesR = pers.tile([1, 128], FP16, tag="onesR")
            ident = pers.tile([128, 128], FP16, tag="ident")
            WkT_sb = pers.tile([128, DK // 128, A], FP16, tag="WkT_sb")
            WqT_sb = pers.tile([128, DQ // 128, A], FP16, tag="WqT_sb")
            kT_sb = pers.tile([128, DK // 128, TK], FP16, tag="kT_sb")
            qT_sb = pers.tile([128, DQ // 128, TQ], FP16, tag="qT_sb")
            value_sb = pers.tile([128, TK // 128, DV], FP16, tag="value_sb")
            y16 = pers.tile([128, TK], FP16, tag="y16")
            rk_sb = pers.tile([128, 4], F32, tag="rk_sb")
            # seeds [x | y]
            shA = pers.tile([128, 1024], FP16, tag="shA")
            qhB = pers.tile([128, 1024], FP16, tag="qhB")
            shB = pers.tile([128, 1024], FP16, tag="shB")
            chB = pers.tile([128, 1024], FP16, tag="chB")
            lev = {h: pers.tile([128, 2048], FP16, name=f"lev_{h}",
                                tag=f"lev_{h}")
                   for h in HKEYS}
            exp_t = [pers.tile([128, TQ], FP16, name=f"exp{g}", tag=f"exp{g}")
                     for g in range(4)]
            rec_row = pers.tile([1, TQ], FP16, tag="rec_row")
            tblw = pers.tile([128, 2], FP16, tag="tblw")

            # ---- input DMA (small chunks spread over queues/sequencers) ----
            kT_re = kT.rearrange("(c p) t -> p c t", p=128)
            qT_re = qT.rearrange("(c p) t -> p c t", p=128)
            val_re = val.rearrange("(c p) d -> p c d", p=128)
            WkT_re = WkT.rearrange("(c p) a -> p c a", p=128)
            WqT_re = WqT.rearrange("(c p) a -> p c a", p=128)
            # sync: consts, WkT+kT-left, then qT c2/c3 left, value
            nc.sync.dma_start(out=cst[:], in_=cst_d[:, :])
            for c in range(4):
                if c != 2:
                    nc.sync.dma_start(out=kT_sb[:, c:c + 1, 0:256],
                                      in_=kT_re[:, c:c + 1, 0:256])
                nc.sync.dma_start(out=WkT_sb[:, c:c + 1, :],
                                  in_=WkT_re[:, c:c + 1, :])
            nc.sync.dma_start(out=qT_sb[:, 2:3, 0:256],
                              in_=qT_re[:, 2:3, 0:256])
            nc.sync.dma_start(out=qT_sb[:, 3:4, 0:256],
                              in_=qT_re[:, 3:4, 0:256])
            nc.sync.dma_start(out=wc[:], in_=wc_d[:, :])
            nc.sync.dma_start(out=value_sb[:, 0:2, :], in_=val_re[:, 0:2, :])
            # scalar: fp16 consts, kT right halves, qT c2/c3 right
            nc.scalar.dma_start(out=cst16[:], in_=cst16_d[:, :])
            for c in range(4):
                if c != 2:
                    nc.scalar.dma_start(out=kT_sb[:, c:c + 1, 256:512],
                                        in_=kT_re[:, c:c + 1, 256:512])
            nc.scalar.dma_start(out=qT_sb[:, 2:3, 256:512],
                                in_=qT_re[:, 2:3, 256:512])
            nc.scalar.dma_start(out=qT_sb[:, 3:4, 256:512],
                                in_=qT_re[:, 3:4, 256:512])
            # gpsimd: kT c2 first (sync/scalar rings are saturated), then
            # WqT pairs and qT c0/c1
            nc.gpsimd.dma_start(out=kT_sb[:, 2:3, 0:256],
                                in_=kT_re[:, 2:3, 0:256])
            nc.gpsimd.dma_start(out=kT_sb[:, 2:3, 256:512],
                                in_=kT_re[:, 2:3, 256:512])
            nc.gpsimd.dma_start(out=WqT_sb[:, 0:2, :], in_=WqT_re[:, 0:2, :])
            for c in range(2):
                nc.gpsimd.dma_start(out=qT_sb[:, c:c + 1, :],
                                    in_=qT_re[:, c:c + 1, :])
            nc.gpsimd.dma_start(out=WqT_sb[:, 2:4, :], in_=WqT_re[:, 2:4, :])
            nc.gpsimd.dma_start(out=onesR[:], in_=onesR_d[:, :])

            # ---- k projection + y seeds ----
            k_ps = psA.tile([128, TK], F32, tag="psA")
            for c in range(DK // 128):
                nc.tensor.matmul(k_ps[:], WkT_sb[:, c, :], kT_sb[:, c, :],
                                 start=(c == 0), stop=(c == DK // 128 - 1))
            nc.vector.tensor_copy(y16[:], k_ps[:])
            # tiny op to pull the Sin table load off the critical path
            nc.scalar.activation(tblw[:, 0:1], cst[:, 4:5], AF.Sin,
                                 bias=0.0, scale=1.0)
            nc.scalar.activation(shA[:, 512:1024], k_ps[:], AF.Sin,
                                 bias=cst[:, 0:1], scale=WA / 2)
            nc.scalar.activation(lev["a1"][:, SY], k_ps[:], AF.Sin,
                                 bias=cst[:, 5:6], scale=WA)
            nc.scalar.activation(qhB[:, 512:1024], k_ps[:], AF.Sin,
                                 bias=cst[:, 2:3], scale=WB / 4)
            nc.scalar.activation(shB[:, 512:1024], k_ps[:], AF.Sin,
                                 bias=cst[:, 3:4], scale=WB / 2)
            # low-priority input DMA behind the seeds on the scalar queue
            nc.scalar.dma_start(out=ident[:], in_=ident_d[:, :])
            nc.scalar.dma_start(out=value_sb[:, 2:4, :], in_=val_re[:, 2:4, :])

            # rk[k] = alpha * (w @ y_proj) as [128k, 1] per k-group
            rk_ps = psA.tile([128, 4], F32, tag="psA")
            for g in range(4):
                nc.tensor.matmul(rk_ps[:, g:g + 1],
                                 y16[:, g * 128:(g + 1) * 128],
                                 cst16[:, 1:2], start=True, stop=True,
                                 skip_group_check=True)
            nc.vector.tensor_copy(rk_sb[:], rk_ps[:])

            # ---- q projection + x seeds (A-family seeds first) ----
            q_ps = psA.tile([128, TQ], F32, tag="psA")
            for c in range(DQ // 128):
                nc.tensor.matmul(q_ps[:], WqT_sb[:, c, :], qT_sb[:, c, :],
                                 start=(c == 0), stop=(c == DQ // 128 - 1))
            nc.scalar.activation(shA[:, 0:512], q_ps[:], AF.Sin,
                                 bias=0.0, scale=WA / 2)
            nc.scalar.activation(lev["a1"][:, SX], q_ps[:], AF.Sin,
                                 bias=0.0, scale=WA)
            nc.scalar.activation(qhB[:, 0:512], q_ps[:], AF.Sin,
                                 bias=0.0, scale=WB / 4)
            nc.scalar.activation(shB[:, 0:512], q_ps[:], AF.Sin,
                                 bias=0.0, scale=WB / 2)

            # dense dummy matmuls to keep the PE busy (HAM warm) while
            # seeds/levels are generated
            dum_ps = psA.tile([128, 64], F32, tag="psA")
            for i in range(12):
                nc.tensor.matmul(dum_ps[:], WkT_sb[:, i % 4, :],
                                 WkT_sb[:, (i + 1) % 4, 0:64],
                                 start=True, stop=True, skip_group_check=True)

            plogs = [psA.tile([128, TQ], F32, name="plog", tag="psA")
                     for _ in range(4)]

            SXH, SYH = slice(0, 512), slice(512, 1024)

            def build_lev1(sl, ssl, csl):
                # family A: sigma1 = sin(wa t) direct (k=1); c1 = 1 - 2 shA^2
                uA = tmp_pool.tile([128, 512], FP16, tag="tmp")
                nc.vector.tensor_tensor(uA[:], shA[:, sl], shA[:, sl], ALU.mult)
                nc.vector.tensor_scalar(
                    lev["a1"][:, csl], uA[:], -2.0, 1.0, ALU.mult, ALU.add)

            def build_lev1B(sl, ssl, csl):
                uB = tmp_pool.tile([128, 512], FP16, tag="tmp")
                nc.vector.tensor_tensor(uB[:], qhB[:, sl], qhB[:, sl], ALU.mult)
                nc.vector.tensor_scalar(
                    chB[:, sl], uB[:], -2.0, 1.0, ALU.mult, ALU.add)
                nc.vector.tensor_tensor(lev["b1"][:, ssl], shB[:, sl],
                                        chB[:, sl], ALU.mult)
                uB2 = tmp_pool.tile([128, 512], FP16, tag="tmp")
                nc.vector.tensor_tensor(uB2[:], shB[:, sl], shB[:, sl],
                                        ALU.mult)
                nc.vector.tensor_scalar(
                    lev["b1"][:, csl], uB2[:], -2.0, 1.0, ALU.mult, ALU.add)

            def dbl(dst, src, ksrc):
                # sigma_2m = sigma_m * c_m ; c_2m = 1 - 2 k^2 sigma_m^2
                nc.vector.tensor_tensor(dst[:, SH], src[:, SH], src[:, CH],
                                        ALU.mult)
                u = tmp_pool.tile([128, 1024], FP16, tag="tmp")
                nc.vector.tensor_tensor(u[:], src[:, SH], src[:, SH], ALU.mult)
                nc.vector.tensor_scalar(
                    dst[:, CH], u[:], -2.0 * ksrc * ksrc, 1.0,
                    ALU.mult, ALU.add)

            def scy_make(h, s_eng, c_eng):
                s = scy_pool.tile([128, 1024], FP16, name=f"scy_{h}", tag="scy")
                hi = HKEYS.index(h)
                for half, src_sl, eng in ((slice(0, 512), SY, s_eng),
                                          (slice(512, 1024), CY, c_eng)):
                    if eng == "S":
                        nc.scalar.activation(s[:, half], lev[h][:, src_sl],
                                             AF.Copy, bias=0.0,
                                             scale=wc[:, hi:hi + 1])
                    elif eng == "G":
                        nc.gpsimd.tensor_scalar_mul(s[:, half],
                                                    lev[h][:, src_sl],
                                                    wc[:, hi:hi + 1])
                    else:
                        nc.vector.tensor_scalar_mul(s[:, half],
                                                    lev[h][:, src_sl],
                                                    wc[:, hi:hi + 1])
                return s

            def products(h, scy_t, first=False, last=False):
                for g in range(4):
                    nc.tensor.matmul(plogs[g][:],
                                     scy_t[:, g * 128:(g + 1) * 128],
                                     lev[h][:, CX], start=first, stop=False)
                    nc.tensor.matmul(plogs[g][:],
                                     scy_t[:, 512 + g * 128:512 + (g + 1) * 128],
                                     lev[h][:, SX], start=False, stop=last)

            # ---- ladder + products, interleaved ----
            build_lev1(SYH, SY, CY)
            for i in range(4):
                nc.tensor.matmul(dum_ps[:], shA[:, 512 + i * 128:640 + i * 128],
                                 shA[:, 512:576], start=True, stop=True,
                                 skip_group_check=True)
            build_lev1(SXH, SX, CX)
            s_a1 = scy_make("a1", "V", "V")
            products("a1", s_a1, first=True)

            dbl(lev["a2"], lev["a1"], 1)
            s_a2 = scy_make("a2", "S", "V")
            products("a2", s_a2)

            build_lev1B(SYH, SY, CY)
            build_lev1B(SXH, SX, CX)
            s_b1 = scy_make("b1", "S", "V")
            products("b1", s_b1)

            dbl(lev["b2"], lev["b1"], 2)
            s_b2 = scy_make("b2", "S", "S")
            products("b2", s_b2)

            dbl(lev["a4"], lev["a2"], 2)
            s_a4 = scy_make("a4", "V", "V")
            products("a4", s_a4)

            dbl(lev["a8"], lev["a4"], 4)
            s_a8 = scy_make("a8", "V", "V")
            products("a8", s_a8)

            dbl(lev["b4"], lev["b2"], 4)
            s_b4 = scy_make("b4", "V", "V")
            # pull the Exp table load off the critical path; reading s_b4
            # (the last scy) pins this late so the Copy table isn't evicted
            # while ScalarE scys still need it
            nc.scalar.activation(tblw[:, 1:2], s_b2[:, 0:1], AF.Exp,
                                 bias=0.0, scale=1.0)
            products("b4", s_b4, last=True)

            # ---- epilogue ----
            for g in range(4):
                nc.scalar.activation(exp_t[g][:], plogs[g][:], AF.Exp,
                                     bias=rk_sb[:, g:g + 1], scale=1.0)

            # out[q, d] = sum_k exp[k, q] value[k, d]; rowsums via ones column
            out_ps = [pout_pool.tile([128, DV], F32, name="out_ps", tag="out")
                      for _ in range(4)]
            rsq_ps = psA.tile([128, 4], F32, tag="psA")
            for gk in range(4):
                for gq in range(4):
                    nc.tensor.matmul(rsq_ps[:, gq:gq + 1],
                                     exp_t[gk][:, gq * 128:(gq + 1) * 128],
                                     cst16[:, 0:1],
                                     start=(gk == 0 and gq == 0),
                                     stop=(gk == 3 and gq == 3),
                                     skip_group_check=True)
                for gq in range(4):
                    nc.tensor.matmul(out_ps[gq][:],
                                     exp_t[gk][:, gq * 128:(gq + 1) * 128],
                                     value_sb[:, gk, :],
                                     start=(gk == 0), stop=(gk == 3))

            # one small reciprocal; broadcast it to a [128, TQ] tile
            recq = epi_pool.tile([128, 4], F32, tag="rec4")
            nc.vector.reciprocal(recq[:, :], rsq_ps[:, :])
            recq16 = epi_pool.tile([128, 4], FP16, tag="rec4")
            nc.vector.tensor_copy(recq16[:], recq[:])
            for g in range(4):
                rT = psA.tile([1, 128], FP16, name=f"recT{g}", tag="psA")
                nc.tensor.transpose(rT[:], recq16[:, g:g + 1], ident[:])
                nc.vector.tensor_copy(rec_row[0:1, g * 128:(g + 1) * 128],
                                      rT[:])
            recB_ps = psA.tile([128, TQ], F32, tag="psA")
            nc.tensor.matmul(recB_ps[:], onesR[:, :], rec_row[0:1, :],
                             start=True, stop=True)
            recB = epi_pool.tile([128, TQ], FP16, tag="recB")
            nc.vector.tensor_copy(recB[:], recB_ps[:])

            # attn tiles (normalized, fp16) + DMA out in halves
            for g in range(4):
                at = epi_pool.tile([128, TQ], FP16, name=f"at{g}", tag="at")
                nc.vector.tensor_tensor(at[:], exp_t[g][:], recB[:], ALU.mult)
                eng = (nc.sync, nc.gpsimd, nc.sync, nc.scalar)[g]
                eng.dma_start(out=attnT_o[g * 128:(g + 1) * 128, 0:256],
                              in_=at[:, 0:256])
                eng2 = (nc.gpsimd, nc.scalar, nc.scalar, nc.sync)[g]
                eng2.dma_start(out=attnT_o[g * 128:(g + 1) * 128, 256:512],
                               in_=at[:, 256:512])

            for gq in range(4):
                ot = epi_pool.tile([128, DV], FP16, name=f"ot{gq}", tag="ot")
                if gq % 2 == 0:
                    nc.vector.tensor_scalar_mul(ot[:], out_ps[gq][:],
                                                recq[:, gq:gq + 1])
                else:
                    nc.scalar.activation(ot[:], out_ps[gq][:], AF.Copy,
                                         bias=0.0, scale=recq[:, gq:gq + 1])
                eng = (nc.gpsimd, nc.scalar, nc.sync, nc.gpsimd)[gq]
                eng.dma_start(out=outN_o[gq * 128:(gq + 1) * 128, 0:256],
                              in_=ot[:, 0:256])
                eng2b = (nc.sync, nc.gpsimd, nc.gpsimd, nc.sync)[gq]
                eng2b.dma_start(out=outN_o[gq * 128:(gq + 1) * 128, 256:512],
                               in_=ot[:, 256:512])



    nc.compile()
    return nc


def _get_nc():
    if "nc" not in _CACHE:
        _CACHE["nc"] = build_nc()
    return _CACHE["nc"]


def make_in_maps(query, key, value, Wq, Wk, bias, w_w, **_):
    coeffs, alpha = _fit_params()
    w = np.asarray(w_w, dtype=np.float64).reshape(A)
    b = np.asarray(bias, dtype=np.float64).reshape(A)
    WqT = np.ascontiguousarray(Wq.T).astype(np.float16)
    WkT = np.ascontiguousarray(Wk.T).astype(np.float16)
    cst = np.zeros((A, 8), dtype=np.float32)
    cst[:, 0] = WA / 2 * b
    cst[:, 1] = WA / 2 * b + HALF_PI
    cst[:, 2] = WB / 4 * b
    cst[:, 3] = WB / 2 * b
    cst[:, 4] = HALF_PI
    cst[:, 5] = WA * b
    cst16 = np.zeros((A, 2), dtype=np.float16)
    cst16[:, 0] = 1.0
    cst16[:, 1] = (alpha * w).astype(np.float16)
    # per-harmonic product scale, with the sigma ladder k_h folded in
    wc = np.zeros((A, 8), dtype=np.float32)
    wc[:, :len(KS)] = (w[:, None]
                       * (coeffs * np.array(KS, np.float64))[None, :])
    onesR = np.ones((1, 128), dtype=np.float16)
    ident = np.eye(128, dtype=np.float16)
    in_maps = []
    for bb in range(B):
        in_maps.append({
            "qT": np.ascontiguousarray(query[bb].T).astype(np.float16),
            "kT": np.ascontiguousarray(key[bb].T).astype(np.float16),
            "value": np.ascontiguousarray(value[bb]).astype(np.float16),
            "WqT": WqT,
            "WkT": WkT,
            "cst": cst,
            "cst16": cst16,
            "wc": wc,
            "onesR": onesR,
            "ident": ident,
        })
    return in_maps


def run(inputs, trace=False, **kwargs):
    nc = _get_nc()
    in_maps = make_in_maps(**{k: np.asarray(v) for k, v in inputs.items()})
    res = run_bass_kernel_spmd(
        nc, in_maps, list(range(N_CORES)), trace=trace, **kwargs
    )
    output = np.stack([res.results[bb]["outN"].astype(np.float32)
                       for bb in range(B)])
    attn = np.stack([
        np.ascontiguousarray(res.results[bb]["attnT"].T).astype(np.float32)
        for bb in range(B)])
    return (output, attn), res


def kernel(**inputs):
    (output, attn), _ = run(inputs)
    return output, attn
